# revision 10
# baseline (speedup 1.0000x reference)
"""Cross-attention Trainium2 kernel (8 NeuronCores, batch-data-parallel).

Computes, per batch element b:
    q = x[b] @ Wq            [S, DK]
    k = y[b] @ Wk            [S, DK]
    v = y[b] @ Wv            [S, E]
    p = exp((q @ k.T) / sqrt(E))        (no max-subtraction: logits ~ N(0, .25))
    out[b] = (p @ v) / rowsum(p) + x[b]

All matmuls run in fp8e4 DoubleRow mode (K=256 per matmul, 2x bf16 rate).
Weights are cast to fp8 unscaled (measured: same end-to-end error as a
16x pre-scale).  The output store is bf16 (upcast to fp32 on the host);
the residual add itself is fp32.

Data movement (per core, BL=2 batches).  The three DMA queues carry
disjoint traffic so the two input tensors stream concurrently:
  - SWDGE (gpsimd): fp32->fp8 D2D casts of the weights and of y into a
    DRAM bounce, plus the xres residual loads.
  - scalar ring: x fp32 reads into SBUF staging (DVE casts them to fp8
    natural-layout tiles, scalar ring stores them to the x bounce),
    weight SBUF loads, output stores.
  - sync ring: xbar pair-transposes only.  A transpose group serializes
    against all in-flight DMAs, so casts pace against the previous
    window and windows are kept to 4 per tensor-batch.
  - Bounces are bitcast to bf16 pairs [S, C/2] and transposed into
    tiles xT[t4] = [128 chan-pairs, S]: partition p of tile t4 holds
    channels (256*t4 + 2p, +1) interleaved along the free dim --
    directly usable as DoubleRow *moving* operands ([128, 2, N],
    strides (1, 2)).
  - DoubleRow *stationary* operands must be pair-blocked (LDWEIGHTS
    rejects a stride-1 pair dim), so yT is additionally deinterleaved
    on DVE into yT_blk [128, 2, S] for the V-projection stationary.

Pipeline (per batch; waves of 512 query columns):
  per half h: qT chunks 2h,2h+1; kT ditto; deint; then v tiles with
  wave-0 scores interleaved (exp lead time).  Per wave w: AV per
  128-query tile with wave w+1's scores interleaved into the PE stream;
  epilogue = DVE (psum * 1/rowsum) + x -> bf16 out.  The next batch's
  input stream (y cast/transpose, x cast/store/transpose, deint) is
  emitted at fixed wave indices so every in-order queue reaches each op
  only after its data has landed.
Engine split: PE matmuls (plus a warmup burst to hold the HAM clock at
2.4 GHz through the ramp); ScalarE exp only; DVE x casts, psum drains,
deinterleave, reciprocal, epilogue.
"""

import math

import numpy as np

# Full-problem constants (hardcoded per the harness contract).
B_FULL = 16
N_CORES = 8
S_Q = 2048
S_KV = 2048
C_DIM = 1024  # input feature dim (contraction of the projections)
DK = 256  # q/k head dim
E_DIM = 1024  # v / output dim
P = 128
QS = 512  # wave size (query cols)
HS = 1024  # stream half size (rows)
N_WARMUP = 32  # PE warmup matmuls


class CFG:
    def __init__(self, bl, sq, skv, c, dk, e):
        assert sq % HS == 0 and skv % HS == 0 and c % 256 == 0 and dk == 256
        self.bl = bl  # batches per core
        self.sq = sq
        self.skv = skv
        self.c = c
        self.dk = dk
        self.e = e
        self.scale = 1.0 / math.sqrt(e)  # exp( (q.k) / sqrt(E) )


def emit_cross_attention(tc, outs, ins, cfg):
    """Emit the kernel into TileContext `tc`.

    ins = x, y, Wq, Wk, Wv ; outs = out.
    x/y: [bl, sq|skv, c] fp32. Weights: [c, dk|e] fp32. out: bf16.
    """
    import concourse.mybir as mybir
    from concourse.mybir import ActivationFunctionType as AF
    from concourse.mybir import AluOpType as ALU
    from concourse.mybir import MatmulPerfMode
    from concourse.tile_rust import add_dep_helper

    nc = tc.nc
    bf16 = mybir.dt.bfloat16
    fp8 = mybir.dt.float8e4
    f32 = mybir.dt.float32
    DR = MatmulPerfMode.DoubleRow

    x, y, Wq, Wk, Wv = ins["x"], ins["y"], ins["Wq"], ins["Wk"], ins["Wv"]
    out = outs["out"]

    nt4 = cfg.c // 256  # channel pair-tiles (256 channels each)
    nt = cfg.skv // P  # key tiles
    nkp = nt // 2  # key pair-tiles
    nd = cfg.dk // P  # dk tiles (2)
    nec = cfg.e // 512  # e chunks
    nw = cfg.sq // QS  # waves
    nhx = cfg.sq // HS  # x stream halves
    nhy = cfg.skv // HS  # y stream halves
    th = HS // P  # key tiles per y half (8)
    gh = HS // P  # 128-row read groups per x half (8)
    mh_w = QS // P  # query tiles per wave (4)

    # DRAM bounce buffers (fp8).
    xb = nc.dram_tensor("xb8", [cfg.bl, cfg.sq, cfg.c], fp8).ap()
    yb = nc.dram_tensor("yb8", [cfg.bl, cfg.skv, cfg.c], fp8).ap()
    wb = {
        "k": nc.dram_tensor("wkb8", [cfg.c, cfg.dk], fp8).ap(),
        "q": nc.dram_tensor("wqb8", [cfg.c, cfg.dk], fp8).ap(),
        "v": nc.dram_tensor("wvb8", [cfg.c, cfg.e], fp8).ap(),
    }
    xb16 = xb.bitcast(bf16)  # [bl, sq, c/2]
    yb16 = yb.bitcast(bf16)

    pool = tc.alloc_tile_pool(name="main", bufs=1)
    ps_mm = tc.alloc_tile_pool(name="ps_mm", bufs=3, space="PSUM")
    ps_av = tc.alloc_tile_pool(name="ps_av", bufs=2, space="PSUM")
    ps_sm = tc.alloc_tile_pool(name="ps_sm", bufs=1, space="PSUM")

    # ---- PE warmup: keep the HAM clock gate open through the DMA ramp ---
    wu = pool.tile([P, QS], fp8, tag="warm", name="warm")
    nc.gpsimd.memset(wu[:], 1.0)
    wu3 = wu[:].rearrange("p (j n) -> p j n", j=2)
    ps_wu = ps_mm.tile([P, QS // 2], f32, tag="mm", name="ps_wu")
    for _ in range(N_WARMUP):
        nc.tensor.matmul(ps_wu[:], wu3[:, :, :P], wu3, start=True, stop=True,
                         perf_mode=DR)
    nc.vector.tensor_copy(wu[:].bitcast(f32)[:, :64], ps_wu[:, :64])

    # ---- weights: SWDGE fp8 cast -> bounce -> [128, 2, M] SBUF loads ----
    def cast_weight(w_dram, name):
        nc.gpsimd.dma_start(out=wb[name], in_=w_dram)

    def load_weight(wdim, t4, name):
        w8 = pool.tile([P, 2, wdim], fp8, tag=f"w8{name}{t4}",
                       name=f"w8{name}{t4}")
        src = wb[name][256 * t4:256 * (t4 + 1), :].rearrange(
            "(p j) m -> p j m", j=2)
        nc.scalar.dma_start(out=w8[:], in_=src)
        return w8

    ones16 = pool.tile([P, 2, 1], fp8, tag="ones", name="ones")
    nc.gpsimd.memset(ones16[:], 1.0)

    # ---- activation stream machinery ------------------------------------
    st = {"last_tg": None}

    def int_view(t):
        # [128, S, 2] fp8 pair-interleaved view of a bf16 transpose tile
        return t[:].bitcast(fp8).rearrange("p (s j) -> p s j", j=2)

    def pace(waiter, dependee):
        if waiter is not None and dependee is not None:
            add_dep_helper(waiter.ins, dependee.ins, sync=True,
                           reason="pace dma windows")

    tiles = {}
    for b in range(cfg.bl):
        for which, n in (("x", cfg.sq), ("y", cfg.skv)):
            tiles[(b, which)] = [
                pool.tile([P, n], bf16, tag=f"{which}T", bufs=2 * nt4,
                          name=f"{which}T{b}_{t4}")
                for t4 in range(nt4)
            ]
        tiles[(b, "yblk")] = [
            pool.tile([P, 2, cfg.skv], fp8, tag="yblk", bufs=2 * nt4,
                      name=f"yblk{b}_{t4}")
            for t4 in range(nt4)
        ]

    def transpose_half(b, which, h):
        dst16 = yb16 if which == "y" else xb16
        ro = h * HS
        tg = None
        for t4 in range(nt4):
            tg = nc.sync.dma_start(
                out=tiles[(b, which)][t4][:, ro:ro + HS],
                in_=dst16[b][ro:ro + HS, t4 * P:(t4 + 1) * P],
                transpose=True,
            )
        st["last_tg"] = tg

    def stream_y_half(b, h):
        ro = h * HS
        c = nc.gpsimd.dma_start(out=yb[b][ro:ro + HS, :],
                                in_=y[b][ro:ro + HS, :])
        pace(c, st["last_tg"])
        transpose_half(b, "y", h)

    def x_half_reads(b, h):
        stg = []
        for g in range(gh):
            ro = h * HS + g * P
            t = pool.tile([P, cfg.c], f32, tag="xstage", bufs=6,
                          name=f"xs{b}_{h}_{g}")
            nc.scalar.dma_start(out=t[:], in_=x[b][ro:ro + P, :])
            stg.append(t)
        st[(b, "xstage", h)] = stg

    def x_half_finish(b, h):
        # DVE cast + scalar store + transpose window for one x half
        stg = st.pop((b, "xstage", h))
        for g in range(gh):
            ro = h * HS + g * P
            t8 = pool.tile([P, cfg.c], fp8, tag="x8n", bufs=4,
                           name=f"x8n{b}_{h}_{g}")
            nc.vector.tensor_copy(t8[:], stg[g][:])
            nc.scalar.dma_start(out=xb[b][ro:ro + P, :], in_=t8[:])
        transpose_half(b, "x", h)

    def deint_half(b, h):
        ro = h * HS
        for t4 in range(nt4):
            nc.vector.tensor_copy(
                tiles[(b, "yblk")][t4][:, :, ro:ro + HS],
                int_view(tiles[(b, "y")][t4])[:, ro:ro + HS, :]
                .transpose([0, 2, 1]),
            )

    # ---- b0 stream section ----------------------------------------------
    cast_weight(Wk, "k")
    cast_weight(Wq, "q")
    wk8 = [load_weight(cfg.dk, t4, "k") for t4 in range(nt4)]
    wq8 = [load_weight(cfg.dk, t4, "q") for t4 in range(nt4)]
    stream_y_half(0, 0)
    x_half_reads(0, 0)
    cast_weight(Wv, "v")
    x_half_finish(0, 0)
    if nhx > 1:
        x_half_reads(0, 1)
    wv8 = [load_weight(cfg.e, t4, "v") for t4 in range(nt4)]
    if nhy > 1:
        stream_y_half(0, 1)
    if nhx > 1:
        x_half_finish(0, 1)
    for b in range(1, cfg.bl):
        for h in range(nhx):
            x_half_reads(b, h)

    # ---- compute ---------------------------------------------------------
    def emit_proj_chunk(b, w8s, which, ci, dst):
        # dst[:, md, ci*QS:...] = (x|y)[chunk ci] @ W  (contraction over c)
        xT = tiles[(b, which)]
        for md in range(nd):
            ps = ps_mm.tile([P, QS], f32, tag="mm", name=f"ps_{which}")
            for t4 in range(nt4):
                mov = int_view(xT[t4])[:, ci * QS:(ci + 1) * QS, :] \
                    .transpose([0, 2, 1])
                nc.tensor.matmul(ps[:], w8s[t4][:, :, md * P:(md + 1) * P],
                                 mov, start=(t4 == 0), stop=(t4 == nt4 - 1),
                                 perf_mode=DR)
            nc.vector.tensor_copy(dst[:, md, ci * QS:(ci + 1) * QS], ps[:])

    def emit_v_tile(b, t, v8):
        yblk = tiles[(b, "yblk")]
        ps_v = ps_av.tile([P, cfg.e], f32, tag="av", name="ps_v")
        for t4 in range(nt4):
            stat = yblk[t4][:, :, t * P:(t + 1) * P]
            for ec in range(nec):
                nc.tensor.matmul(ps_v[:, 512 * ec:512 * (ec + 1)],
                                 stat, wv8[t4][:, :, 512 * ec:512 * (ec + 1)],
                                 start=(t4 == 0), stop=(t4 == nt4 - 1),
                                 perf_mode=DR)
        nc.vector.tensor_copy(v8[:, t, :], ps_v[:])

    def emit_score(kT8, qT8, wo, t, pT_w):
        # one key-tile's scores for wave at query offset wo, plus exp
        ps = ps_mm.tile([P, QS], f32, tag="mm", name="ps_s")
        nc.tensor.matmul(ps[:], kT8[:, :, t * P:(t + 1) * P],
                         qT8[:, :, wo:wo + QS], start=True, stop=True,
                         perf_mode=DR)
        nc.scalar.activation(pT_w[t // 2][:, t % 2, :], ps[:], AF.Exp,
                             scale=cfg.scale)

    def make_pT():
        return [
            pool.tile([P, 2, QS], fp8, tag="pT", bufs=2 * nkp,
                      name=f"pT{kp}")
            for kp in range(nkp)
        ]

    for b in range(cfg.bl):
        kT8 = pool.tile([P, nd, cfg.skv], fp8, tag="kT", bufs=2, name="kT")
        qT8 = pool.tile([P, nd, cfg.sq], fp8, tag="qT", bufs=2, name="qT")
        v8 = pool.tile([P, nt, cfg.e], fp8, tag="v8", bufs=1, name="v8")

        pT_cur = make_pT()
        # half-granular projections, with wave-0 scores interleaved into
        # the v-tile loop so exp gets lead time before AV.
        for h in range(max(nhx, nhy)):
            if h < nhx:
                emit_proj_chunk(b, wq8, "x", 2 * h, qT8)
                emit_proj_chunk(b, wq8, "x", 2 * h + 1, qT8)
            if h < nhy:
                emit_proj_chunk(b, wk8, "y", 2 * h, kT8)
                emit_proj_chunk(b, wk8, "y", 2 * h + 1, kT8)
                if b == 0:
                    deint_half(b, h)
                for t in range(th * h, th * (h + 1)):
                    emit_v_tile(b, t, v8)
                    emit_score(kT8, qT8, 0, t, pT_cur)

        # waves
        for w in range(nw):
            wo = w * QS
            pT_next = make_pT() if w + 1 < nw else None
            # next batch's input stream, spread across this batch's waves
            bn = b + 1
            if bn < cfg.bl:
                if w == 0:
                    stream_y_half(bn, 0)
                elif w == 1:
                    x_half_finish(bn, 0)
                    if nhy > 1:
                        stream_y_half(bn, 1)
                elif w == 2:
                    if nhx > 1:
                        x_half_finish(bn, 1)
                    deint_half(bn, 0)
                elif w == 3:
                    if nhy > 1:
                        deint_half(bn, 1)
            for mh in range(mh_w):
                sm = wo + mh * P
                ps_e = ps_av.tile([P, cfg.e], f32, tag="av", name="ps_e")
                ps_sum = ps_sm.tile([P, 1], f32, tag="sum", name="ps_sum")
                for kp in range(nkp):
                    stat = pT_cur[kp][:, :, mh * P:(mh + 1) * P]
                    for ec in range(nec):
                        nc.tensor.matmul(
                            ps_e[:, 512 * ec:512 * (ec + 1)],
                            stat, v8[:, 2 * kp:2 * kp + 2,
                                     512 * ec:512 * (ec + 1)],
                            start=(kp == 0), stop=(kp == nkp - 1),
                            perf_mode=DR)
                    nc.tensor.matmul(ps_sum[:], stat, ones16[:],
                                     start=(kp == 0), stop=(kp == nkp - 1),
                                     perf_mode=DR)
                # interleave next wave's scores into the PE stream
                if pT_next is not None:
                    npm = nt // mh_w
                    for t in range(mh * npm, (mh + 1) * npm):
                        emit_score(kT8, qT8, wo + QS, t, pT_next)

                recip = pool.tile([P, 1], f32, tag="recip", bufs=4,
                                  name="recip")
                nc.vector.reciprocal(recip[:], ps_sum[:])
                xres = pool.tile([P, cfg.e], f32, tag="xres", bufs=2,
                                 name="xres")
                nc.gpsimd.dma_start(out=xres[:], in_=x[b][sm:sm + P, :])
                out_t = pool.tile([P, cfg.e], bf16, tag="out_t", bufs=3,
                                  name="out_t")
                nc.vector.scalar_tensor_tensor(
                    out_t[:], ps_e[:], recip[:], xres[:], ALU.mult, ALU.add)
                nc.scalar.dma_start(out=out[b][sm:sm + P, :], in_=out_t[:])
            pT_cur = pT_next

    ps_sm.release()
    ps_av.release()
    ps_mm.release()
    pool.release()


def make_tile_kernel(cfg):
    """Adapter with the (tc, outs, ins) signature used by run_kernel/test.py."""

    def k(tc, outs, ins):
        emit_cross_attention(tc, outs, ins, cfg)

    return k


def _build(cfg):
    import concourse.bacc as bacc
    import concourse.mybir as mybir
    import concourse.tile as tile

    f32 = mybir.dt.float32
    bf16 = mybir.dt.bfloat16
    nc = bacc.Bacc(
        "TRN2",
        target_bir_lowering=False,
        debug=False,
        enable_asserts=False,
        num_devices=N_CORES,
    )
    ins = {
        "x": nc.dram_tensor("x", [cfg.bl, cfg.sq, cfg.c], f32, kind="ExternalInput").ap(),
        "y": nc.dram_tensor("y", [cfg.bl, cfg.skv, cfg.c], f32, kind="ExternalInput").ap(),
        "Wq": nc.dram_tensor("Wq", [cfg.c, cfg.dk], f32, kind="ExternalInput").ap(),
        "Wk": nc.dram_tensor("Wk", [cfg.c, cfg.dk], f32, kind="ExternalInput").ap(),
        "Wv": nc.dram_tensor("Wv", [cfg.c, cfg.e], f32, kind="ExternalInput").ap(),
    }
    outs = {
        "out": nc.dram_tensor("out", [cfg.bl, cfg.sq, cfg.e], bf16, kind="ExternalOutput").ap()
    }
    with tile.TileContext(nc) as tc:
        emit_cross_attention(tc, outs, ins, cfg)
    nc.compile()
    return nc


_CACHED = {}


def run_on_cores(x, y, Wq, Wk, Wv, trace=False):
    from concourse import bass_utils

    cfg = CFG(B_FULL // N_CORES, S_Q, S_KV, C_DIM, DK, E_DIM)
    key = "full"
    if key not in _CACHED:
        _CACHED[key] = _build(cfg)
    nc = _CACHED[key]

    bl = cfg.bl
    in_maps = [
        {
            "x": np.ascontiguousarray(x[i * bl : (i + 1) * bl]),
            "y": np.ascontiguousarray(y[i * bl : (i + 1) * bl]),
            "Wq": Wq,
            "Wk": Wk,
            "Wv": Wv,
        }
        for i in range(N_CORES)
    ]
    res = bass_utils.run_bass_kernel_spmd(
        nc, in_maps, core_ids=list(range(N_CORES)), trace=trace
    )
    out = np.concatenate(
        [np.asarray(r["out"]).astype(np.float32) for r in res.results], axis=0
    )
    return out, res


def kernel(x, y, Wq, Wk, Wv):
    x = np.asarray(x, dtype=np.float32)
    y = np.asarray(y, dtype=np.float32)
    Wq = np.asarray(Wq, dtype=np.float32)
    Wk = np.asarray(Wk, dtype=np.float32)
    Wv = np.asarray(Wv, dtype=np.float32)
    out, _ = run_on_cores(x, y, Wq, Wk, Wv, trace=False)
    return out


# revision 11
# speedup vs baseline: 1.1954x; 1.1954x over previous
"""Cross-attention Trainium2 kernel (8 NeuronCores, batch-data-parallel).

Computes, per batch element b:
    q = x[b] @ Wq            [S, DK]
    k = y[b] @ Wk            [S, DK]
    v = y[b] @ Wv            [S, E]
    p = exp((q @ k.T) / sqrt(E))        (no max-subtraction: logits ~ N(0, .25))
    out[b] = (p @ v) / rowsum(p) + x[b]

All matmuls run in fp8e4 DoubleRow mode (K=256 per matmul, 2x bf16 rate).
Weights are cast to fp8 by SWDGE D2D casts, unscaled (measured: same
end-to-end error as a 16x pre-scale).  The output store is bf16 (upcast
to fp32 on the host); the residual add itself is fp32.

Data movement (per core, BL=2 batches):
  - SWDGE (gpsimd queue): fp32->fp8 D2D casts -- weights into weight
    bounces, x and y in 1024-row halves into activation bounces.  One
    monolithic cast per half: multi-hop alternatives add ~15us of DMA
    pipeline latency per hop and starve downstream consumers.
  - sync ring: xbar pair-transposes only.  A transpose group serializes
    against all in-flight DMAs, so casts pace against the previous
    window and windows are kept to 4 per tensor-batch.
  - scalar ring: weight SBUF loads, xres residual loads, output stores.
  - Bounces are bitcast to bf16 pairs [S, C/2] and transposed into
    tiles xT[t4] = [128 chan-pairs, S]: partition p of tile t4 holds
    channels (256*t4 + 2p, +1) interleaved along the free dim --
    directly usable as DoubleRow *moving* operands ([128, 2, N],
    strides (1, 2)).
  - DoubleRow *stationary* operands must be pair-blocked (LDWEIGHTS
    rejects a stride-1 pair dim), so yT is additionally deinterleaved
    on DVE into yT_blk [128, 2, S] for the V-projection stationary.

Pipeline (per batch; waves of 512 query columns):
  per half h: qT chunks 2h,2h+1; kT ditto; deint; then v tiles with
  wave-0 scores interleaved (exp lead time).  Per wave w: AV per
  128-query tile with wave w+1's scores interleaved into the PE stream;
  epilogue = DVE (psum * 1/rowsum) + x -> bf16 out.  The next batch's
  input stream and deinterleave are emitted at fixed wave indices so
  every in-order queue reaches each op only after its data has landed.
Engine split: PE matmuls (plus a warmup burst to hold the HAM clock at
2.4 GHz through the ramp); ScalarE exp only; DVE psum drains,
deinterleave, reciprocal, epilogue.
"""

import math

import numpy as np

# Full-problem constants (hardcoded per the harness contract).
B_FULL = 16
N_CORES = 8
S_Q = 2048
S_KV = 2048
C_DIM = 1024  # input feature dim (contraction of the projections)
DK = 256  # q/k head dim
E_DIM = 1024  # v / output dim
P = 128
QS = 512  # wave size (query cols)
HS = 1024  # stream half size (rows)
N_WARMUP = 40  # PE warmup matmuls


class CFG:
    def __init__(self, bl, sq, skv, c, dk, e):
        assert sq % HS == 0 and skv % HS == 0 and c % 256 == 0 and dk == 256
        self.bl = bl  # batches per core
        self.sq = sq
        self.skv = skv
        self.c = c
        self.dk = dk
        self.e = e
        self.scale = 1.0 / math.sqrt(e)  # exp( (q.k) / sqrt(E) )


def emit_cross_attention(tc, outs, ins, cfg):
    """Emit the kernel into TileContext `tc`.

    ins = x, y, Wq, Wk, Wv ; outs = out.
    x/y: [bl, sq|skv, c] fp32. Weights: [c, dk|e] fp32. out: bf16.
    """
    import concourse.mybir as mybir
    from concourse.mybir import ActivationFunctionType as AF
    from concourse.mybir import AluOpType as ALU
    from concourse.mybir import MatmulPerfMode
    from concourse.tile_rust import add_dep_helper

    nc = tc.nc
    bf16 = mybir.dt.bfloat16
    fp8 = mybir.dt.float8e4
    f32 = mybir.dt.float32
    DR = MatmulPerfMode.DoubleRow

    x, y, Wq, Wk, Wv = ins["x"], ins["y"], ins["Wq"], ins["Wk"], ins["Wv"]
    out = outs["out"]

    nt4 = cfg.c // 256  # channel pair-tiles (256 channels each)
    nt = cfg.skv // P  # key tiles
    nkp = nt // 2  # key pair-tiles
    nd = cfg.dk // P  # dk tiles (2)
    nec = cfg.e // 512  # e chunks
    nw = cfg.sq // QS  # waves
    nhx = cfg.sq // HS  # x stream halves
    nhy = cfg.skv // HS  # y stream halves
    th = HS // P  # key tiles per y half (8)
    mh_w = QS // P  # query tiles per wave (4)

    # DRAM bounce buffers (fp8).
    xb = nc.dram_tensor("xb8", [cfg.bl, cfg.sq, cfg.c], fp8).ap()
    yb = nc.dram_tensor("yb8", [cfg.bl, cfg.skv, cfg.c], fp8).ap()
    wb = {
        "k": nc.dram_tensor("wkb8", [cfg.c, cfg.dk], fp8).ap(),
        "q": nc.dram_tensor("wqb8", [cfg.c, cfg.dk], fp8).ap(),
        "v": nc.dram_tensor("wvb8", [cfg.c, cfg.e], fp8).ap(),
    }
    xb16 = xb.bitcast(bf16)  # [bl, sq, c/2]
    yb16 = yb.bitcast(bf16)

    pool = tc.alloc_tile_pool(name="main", bufs=1)
    ps_mm = tc.alloc_tile_pool(name="ps_mm", bufs=3, space="PSUM")
    ps_av = tc.alloc_tile_pool(name="ps_av", bufs=2, space="PSUM")
    ps_sm = tc.alloc_tile_pool(name="ps_sm", bufs=1, space="PSUM")

    # ---- PE warmup: keep the HAM clock gate open through the DMA ramp ---
    wu = pool.tile([P, QS], fp8, tag="warm", name="warm")
    nc.gpsimd.memset(wu[:], 1.0)
    wu3 = wu[:].rearrange("p (j n) -> p j n", j=2)
    ps_wu = ps_mm.tile([P, QS // 2], f32, tag="mm", name="ps_wu")
    for _ in range(N_WARMUP):
        nc.tensor.matmul(ps_wu[:], wu3[:, :, :P], wu3, start=True, stop=True,
                         perf_mode=DR)
    nc.vector.tensor_copy(wu[:].bitcast(f32)[:, :64], ps_wu[:, :64])

    # ---- weights: SWDGE fp8 cast -> bounce -> [128, 2, M] SBUF loads ----
    def load_weight(wdim, t4, name):
        w8 = pool.tile([P, 2, wdim], fp8, tag=f"w8{name}{t4}",
                       name=f"w8{name}{t4}")
        src = wb[name][256 * t4:256 * (t4 + 1), :].rearrange(
            "(p j) m -> p j m", j=2)
        nc.scalar.dma_start(out=w8[:], in_=src)
        return w8

    ones16 = pool.tile([P, 2, 1], fp8, tag="ones", name="ones")
    nc.gpsimd.memset(ones16[:], 1.0)

    # ---- activation stream machinery ------------------------------------
    st = {"last_tg": None}

    def int_view(t):
        # [128, S, 2] fp8 pair-interleaved view of a bf16 transpose tile
        return t[:].bitcast(fp8).rearrange("p (s j) -> p s j", j=2)

    def pace(waiter, dependee):
        if waiter is not None and dependee is not None:
            add_dep_helper(waiter.ins, dependee.ins, sync=True,
                           reason="pace dma windows")

    tiles = {}
    for b in range(cfg.bl):
        for which, n in (("x", cfg.sq), ("y", cfg.skv)):
            tiles[(b, which)] = [
                pool.tile([P, n], bf16, tag=f"{which}T", bufs=2 * nt4,
                          name=f"{which}T{b}_{t4}")
                for t4 in range(nt4)
            ]
        tiles[(b, "yblk")] = [
            pool.tile([P, 2, cfg.skv], fp8, tag="yblk", bufs=2 * nt4,
                      name=f"yblk{b}_{t4}")
            for t4 in range(nt4)
        ]

    def stream_half(b, which, h):
        src = y if which == "y" else x
        dstb = yb if which == "y" else xb
        dst16 = yb16 if which == "y" else xb16
        ro = h * HS
        c = nc.gpsimd.dma_start(out=dstb[b][ro:ro + HS, :],
                                in_=src[b][ro:ro + HS, :])
        pace(c, st["last_tg"])
        tg = None
        for t4 in range(nt4):
            tg = nc.sync.dma_start(
                out=tiles[(b, which)][t4][:, ro:ro + HS],
                in_=dst16[b][ro:ro + HS, t4 * P:(t4 + 1) * P],
                transpose=True,
            )
        st["last_tg"] = tg

    def deint_half(b, h):
        ro = h * HS
        for t4 in range(nt4):
            nc.vector.tensor_copy(
                tiles[(b, "yblk")][t4][:, :, ro:ro + HS],
                int_view(tiles[(b, "y")][t4])[:, ro:ro + HS, :]
                .transpose([0, 2, 1]),
            )

    # ---- b0 stream section ----------------------------------------------
    nc.gpsimd.dma_start(out=wb["k"], in_=Wk)
    nc.gpsimd.dma_start(out=wb["q"], in_=Wq)
    wk8 = [load_weight(cfg.dk, t4, "k") for t4 in range(nt4)]
    wq8 = [load_weight(cfg.dk, t4, "q") for t4 in range(nt4)]
    stream_half(0, "y", 0)
    nc.gpsimd.dma_start(out=wb["v"], in_=Wv)
    stream_half(0, "x", 0)
    wv8 = [load_weight(cfg.e, t4, "v") for t4 in range(nt4)]
    if nhy > 1:
        stream_half(0, "y", 1)
    if nhx > 1:
        stream_half(0, "x", 1)

    # ---- compute ---------------------------------------------------------
    def emit_proj_half(b, w8s, which, h, dst):
        # dst[:, md, h*HS:...] = (x|y)[half h] @ W  (contraction over c);
        # two 512-chunks per stationary so LDWEIGHTS amortizes.
        xT = tiles[(b, which)]
        for md in range(nd):
            pss = [ps_mm.tile([P, QS], f32, tag="mm", name=f"ps_{which}")
                   for _ in range(2)]
            for t4 in range(nt4):
                stat = w8s[t4][:, :, md * P:(md + 1) * P]
                for ci in range(2):
                    co = h * HS + ci * QS
                    mov = int_view(xT[t4])[:, co:co + QS, :] \
                        .transpose([0, 2, 1])
                    nc.tensor.matmul(pss[ci][:], stat, mov,
                                     start=(t4 == 0), stop=(t4 == nt4 - 1),
                                     perf_mode=DR)
            for ci in range(2):
                co = h * HS + ci * QS
                nc.vector.tensor_copy(dst[:, md, co:co + QS], pss[ci][:])

    def emit_v_tile(b, t, v8):
        yblk = tiles[(b, "yblk")]
        ps_v = ps_av.tile([P, cfg.e], f32, tag="av", name="ps_v")
        for t4 in range(nt4):
            stat = yblk[t4][:, :, t * P:(t + 1) * P]
            for ec in range(nec):
                nc.tensor.matmul(ps_v[:, 512 * ec:512 * (ec + 1)],
                                 stat, wv8[t4][:, :, 512 * ec:512 * (ec + 1)],
                                 start=(t4 == 0), stop=(t4 == nt4 - 1),
                                 perf_mode=DR)
        nc.vector.tensor_copy(v8[:, t, :], ps_v[:])

    def emit_score(kT8, qT8, wo, t, pT_w):
        # one key-tile's scores for wave at query offset wo, plus exp
        ps = ps_mm.tile([P, QS], f32, tag="mm", name="ps_s")
        nc.tensor.matmul(ps[:], kT8[:, :, t * P:(t + 1) * P],
                         qT8[:, :, wo:wo + QS], start=True, stop=True,
                         perf_mode=DR)
        nc.scalar.activation(pT_w[t // 2][:, t % 2, :], ps[:], AF.Exp,
                             scale=cfg.scale)

    def make_pT():
        return [
            pool.tile([P, 2, QS], fp8, tag="pT", bufs=2 * nkp,
                      name=f"pT{kp}")
            for kp in range(nkp)
        ]

    for b in range(cfg.bl):
        kT8 = pool.tile([P, nd, cfg.skv], fp8, tag="kT", bufs=2, name="kT")
        qT8 = pool.tile([P, nd, cfg.sq], fp8, tag="qT", bufs=2, name="qT")
        v8 = pool.tile([P, nt, cfg.e], fp8, tag="v8", bufs=1, name="v8")

        pT_cur = make_pT()
        # half-granular projections, with wave-0 scores interleaved into
        # the v-tile loop so exp gets lead time before AV.
        for h in range(max(nhx, nhy)):
            if h < nhx:
                emit_proj_half(b, wq8, "x", h, qT8)
            if h < nhy:
                emit_proj_half(b, wk8, "y", h, kT8)
                if b == 0:
                    deint_half(b, h)
                for t in range(th * h, th * (h + 1)):
                    emit_v_tile(b, t, v8)
                    emit_score(kT8, qT8, 0, t, pT_cur)

        # waves
        for w in range(nw):
            wo = w * QS
            pT_next = make_pT() if w + 1 < nw else None
            # next batch's input stream, spread across this batch's waves
            bn = b + 1
            if bn < cfg.bl:
                if w == 0:
                    stream_half(bn, "y", 0)
                elif w == 1:
                    stream_half(bn, "x", 0)
                    if nhy > 1:
                        stream_half(bn, "y", 1)
                elif w == 2:
                    if nhx > 1:
                        stream_half(bn, "x", 1)
                    deint_half(bn, 0)
                elif w == 3:
                    if nhy > 1:
                        deint_half(bn, 1)
            for mh in range(mh_w):
                sm = wo + mh * P
                ps_e = ps_av.tile([P, cfg.e], f32, tag="av", name="ps_e")
                ps_sum = ps_sm.tile([P, 1], f32, tag="sum", name="ps_sum")
                for kp in range(nkp):
                    stat = pT_cur[kp][:, :, mh * P:(mh + 1) * P]
                    for ec in range(nec):
                        nc.tensor.matmul(
                            ps_e[:, 512 * ec:512 * (ec + 1)],
                            stat, v8[:, 2 * kp:2 * kp + 2,
                                     512 * ec:512 * (ec + 1)],
                            start=(kp == 0), stop=(kp == nkp - 1),
                            perf_mode=DR)
                    nc.tensor.matmul(ps_sum[:], stat, ones16[:],
                                     start=(kp == 0), stop=(kp == nkp - 1),
                                     perf_mode=DR)
                # interleave next wave's scores into the PE stream
                if pT_next is not None:
                    npm = nt // mh_w
                    for t in range(mh * npm, (mh + 1) * npm):
                        emit_score(kT8, qT8, wo + QS, t, pT_next)

                recip = pool.tile([P, 1], f32, tag="recip", bufs=4,
                                  name="recip")
                nc.vector.reciprocal(recip[:], ps_sum[:])
                xres = pool.tile([P, cfg.e], f32, tag="xres", bufs=2,
                                 name="xres")
                nc.scalar.dma_start(out=xres[:], in_=x[b][sm:sm + P, :])
                out_t = pool.tile([P, cfg.e], bf16, tag="out_t", bufs=3,
                                  name="out_t")
                nc.vector.scalar_tensor_tensor(
                    out_t[:], ps_e[:], recip[:], xres[:], ALU.mult, ALU.add)
                nc.scalar.dma_start(out=out[b][sm:sm + P, :], in_=out_t[:])
            pT_cur = pT_next

    ps_sm.release()
    ps_av.release()
    ps_mm.release()
    pool.release()


def make_tile_kernel(cfg):
    """Adapter with the (tc, outs, ins) signature used by run_kernel/test.py."""

    def k(tc, outs, ins):
        emit_cross_attention(tc, outs, ins, cfg)

    return k


def _build(cfg):
    import concourse.bacc as bacc
    import concourse.mybir as mybir
    import concourse.tile as tile

    f32 = mybir.dt.float32
    bf16 = mybir.dt.bfloat16
    nc = bacc.Bacc(
        "TRN2",
        target_bir_lowering=False,
        debug=False,
        enable_asserts=False,
        num_devices=N_CORES,
    )
    ins = {
        "x": nc.dram_tensor("x", [cfg.bl, cfg.sq, cfg.c], f32, kind="ExternalInput").ap(),
        "y": nc.dram_tensor("y", [cfg.bl, cfg.skv, cfg.c], f32, kind="ExternalInput").ap(),
        "Wq": nc.dram_tensor("Wq", [cfg.c, cfg.dk], f32, kind="ExternalInput").ap(),
        "Wk": nc.dram_tensor("Wk", [cfg.c, cfg.dk], f32, kind="ExternalInput").ap(),
        "Wv": nc.dram_tensor("Wv", [cfg.c, cfg.e], f32, kind="ExternalInput").ap(),
    }
    outs = {
        "out": nc.dram_tensor("out", [cfg.bl, cfg.sq, cfg.e], bf16, kind="ExternalOutput").ap()
    }
    with tile.TileContext(nc) as tc:
        emit_cross_attention(tc, outs, ins, cfg)
    nc.compile()
    return nc


_CACHED = {}


def run_on_cores(x, y, Wq, Wk, Wv, trace=False):
    from concourse import bass_utils

    cfg = CFG(B_FULL // N_CORES, S_Q, S_KV, C_DIM, DK, E_DIM)
    key = "full"
    if key not in _CACHED:
        _CACHED[key] = _build(cfg)
    nc = _CACHED[key]

    bl = cfg.bl
    in_maps = [
        {
            "x": np.ascontiguousarray(x[i * bl : (i + 1) * bl]),
            "y": np.ascontiguousarray(y[i * bl : (i + 1) * bl]),
            "Wq": Wq,
            "Wk": Wk,
            "Wv": Wv,
        }
        for i in range(N_CORES)
    ]
    res = bass_utils.run_bass_kernel_spmd(
        nc, in_maps, core_ids=list(range(N_CORES)), trace=trace
    )
    out = np.concatenate(
        [np.asarray(r["out"]).astype(np.float32) for r in res.results], axis=0
    )
    return out, res


def kernel(x, y, Wq, Wk, Wv):
    x = np.asarray(x, dtype=np.float32)
    y = np.asarray(y, dtype=np.float32)
    Wq = np.asarray(Wq, dtype=np.float32)
    Wk = np.asarray(Wk, dtype=np.float32)
    Wv = np.asarray(Wv, dtype=np.float32)
    out, _ = run_on_cores(x, y, Wq, Wk, Wv, trace=False)
    return out
